# revision 6
# baseline (speedup 1.0000x reference)
"""Trainium2 Bass kernel for the exp-kernel multivariate Hawkes process
log-likelihood (B=8, N=2048, D=10).

Strategy (v5)
-------------
Data-parallel over batch: core b computes batch row b fully on-chip and
returns pev[128,16] per-event partials; the host reduces them and adds
the -T*sum(mu) constant (unshard step).

Host ships per-event GATHERED tables (pure index lookups, O(N*D)):
  bcol[j,r]  = b[r, e_j]        brow_neg[j,m] = -b[e_j, m]
  acolT[j,d] = a[d, e_j]        abrow[j,m]    = (a*b)[e_j, m]
so the device only exponentiates [N,10] grids:
  u[j,r]    = exp(bcol * trel_j)       expn = exp(bcol * tau2)
  vab[i,m]  = exp(brow_neg * trel_i) * abrow
The u exp and the [128, KC*100] pair grid W = u (x) onehot_m are built
in halves so the prefix matmuls start as early as possible.  The
secondary products (varg, vab, nmul) run on the otherwise-idle GPSIMD
engine so they can never head-of-line-block the DVE critical chain.

Tail (factored, no second pair grid):
  lam_core[i] = sum_r onehot_r[i,r] * (sum_m PgSB[i,(r,m)] * vab[i,m])
via per-group 2x products against Act-engine bf16 copies of the PSUM
prefix + X-reduces, pipelined group-by-group against the copies.  The
inter-chunk S contribution and the r-mask share one concatenated
[P,KC,2,D] mask pass.

Inter-chunk state S lives in [10_r, (10_m, 16_k)] layout end-to-end:
chunk sums from 16 tiny u^T@onehot matmuls (strided PSUM writes), the
affine recurrence S_{k+1}=d_k(S_k+w_k) is ONE tensor_tensor_scan with
a k=0 decay-reset column, and S is gathered per event with 15 onehotT
matmuls.  A manually emitted InstLoadActFuncSet(id=6) loads the
combined exp+ln table once.

Host-side work is limited to O(D^2) parameter softplus and O(N*D)
gathers/reshapes/sums of index tensors (no transcendental math on N).
"""
import numpy as np
from contextlib import ExitStack

import ml_dtypes
import concourse.bass as bass
import concourse.mybir as mybir
import concourse.tile as tile
from concourse import bacc
from concourse.bass_utils import run_bass_kernel_spmd

f32 = mybir.dt.float32
bf16 = mybir.dt.bfloat16
AL = mybir.AluOpType
AF = mybir.ActivationFunctionType
AX = mybir.AxisListType

P = 128          # partitions == chunk size
KC = 16          # number of chunks
D = 10           # event types
RM = D * D       # (receiver, trigger) pairs
N = P * KC       # 2048 events per batch row
B = 8            # batch == cores
NG = 4           # chunk groups (4 chunks per PSUM bank)

# packed DRAM inputs: name -> (shape, dtype)
INPUTS = {
    "hot_f32": ((P, 192), f32),    # trel(16) tau2(16) bcol(160)
    "hot_bf": ((P, 288), bf16),    # onehot(160) triu(128)
    "rest_f32": ((P, 208), f32),   # brow_neg(160) musub_ev(16) asum_ev(16)
                                   # pad(16)
    "rest_bf": ((P, 640), bf16),   # acolT(160) abrow(160) ohone(320)
    "oht": ((D, N + 320), bf16),   # onehotT [D,N] | decay args true |
                                   # decay args k0-killed
}


def _body(ctx: ExitStack, tc, ins, out_ap):
    nc = tc.nc
    cpool = ctx.enter_context(tc.tile_pool(name="cpool", bufs=1))
    wpool = ctx.enter_context(tc.tile_pool(name="wpool", bufs=1))
    pp = ctx.enter_context(tc.tile_pool(name="pp", bufs=1, space="PSUM"))

    # one combined exp+ln activation table load (id 6 =
    # natural_log_exp_and_others) emitted first on the Act queue
    nc.scalar.add_instruction(mybir.InstLoadActFuncSet(
        name=nc.get_next_instruction_name(), act_func_set_id=6,
        ins=[], outs=[]))

    # ---- input DMAs: hot tiles first on the sync/scalar queues ----
    hot_f32 = cpool.tile([P, 192], f32, tag="hot_f32")
    nc.sync.dma_start(out=hot_f32[:], in_=ins["hot_f32"])
    hot_bf = cpool.tile([P, 288], bf16, tag="hot_bf")
    nc.scalar.dma_start(out=hot_bf[:], in_=ins["hot_bf"])
    rest_f32 = cpool.tile([P, 208], f32, tag="rest_f32")
    nc.sync.dma_start(out=rest_f32[:], in_=ins["rest_f32"])
    rest_bf = cpool.tile([P, 640], bf16, tag="rest_bf")
    nc.scalar.dma_start(out=rest_bf[:], in_=ins["rest_bf"])
    oht = cpool.tile([D, N + 320], bf16, tag="oht")
    nc.gpsimd.dma_start(out=oht[:], in_=ins["oht"])

    trel = hot_f32[:, 0:16]
    tau2 = hot_f32[:, 16:32]
    bcol = hot_f32[:, 32:192].rearrange("p (c r) -> p c r", c=KC)
    onehot = hot_bf[:, 0:160].rearrange("p (c m) -> p c m", c=KC)
    triu = hot_bf[:, 160:288]
    brow_neg = rest_f32[:, 0:160].rearrange("p (c m) -> p c m", c=KC)
    musub_ev = rest_f32[:, 160:176]
    asum_ev = rest_f32[:, 176:192]
    acolT = rest_bf[:, 0:160].rearrange("p (c d) -> p c d", c=KC)
    abrow = rest_bf[:, 160:320].rearrange("p (c m) -> p c m", c=KC)
    ohone = rest_bf[:, 320:640].rearrange("p (c s m) -> p c s m", c=KC, s=2)
    bdtb_true = oht[:, N:N + 160].rearrange("p (m k) -> p m k", m=D)
    bdtb_k0 = oht[:, N + 160:N + 320]

    # ---- critical chain: au -> eu (halves) -> W (halves) -> prefix ----
    au = wpool.tile([P, KC, D], f32, tag="au")
    nc.vector.tensor_tensor(
        out=au[:], in0=bcol,
        in1=trel.unsqueeze(2).broadcast_to([P, KC, D]), op=AL.mult)
    u = wpool.tile([P, KC, D], bf16, tag="u")
    nc.scalar.activation(u[:, 0:8], au[:, 0:8], AF.Exp)
    nc.scalar.activation(u[:, 8:16], au[:, 8:16], AF.Exp)
    W = wpool.tile([P, KC, D, D], bf16, tag="W")
    for h in range(2):
        hs = slice(8 * h, 8 * (h + 1))
        nc.vector.tensor_tensor(
            out=W[:, hs],
            in0=u[:, hs].unsqueeze(3).broadcast_to([P, 8, D, D]),
            in1=onehot[:, hs].unsqueeze(2).broadcast_to([P, 8, D, D]),
            op=AL.mult)

    # negative-part exp arg (DVE bubble while Act runs eu halves)
    an = wpool.tile([P, KC, D], f32, tag="an")
    nc.vector.tensor_tensor(
        out=an[:], in0=bcol,
        in1=tau2.unsqueeze(2).broadcast_to([P, KC, D]), op=AL.mult)
    expn = wpool.tile([P, KC, D], bf16, tag="expn")
    nc.scalar.activation(expn[:], an[:], AF.Exp)

    # chunk sums straight into scan layout: wsq[r, m, k]
    wsq = pp.tile([D, D, KC], f32, tag="wsq", name="wsq")
    for k in range(KC):
        nc.tensor.matmul(wsq[:, :, k], u[:, k, :], onehot[:, k, :],
                         start=True, stop=True)

    # ---- secondary exp pipeline: products on GPSIMD ----
    varg = wpool.tile([P, KC, D], f32, tag="varg")
    nc.gpsimd.tensor_tensor(
        out=varg[:], in0=brow_neg,
        in1=trel.unsqueeze(2).broadcast_to([P, KC, D]), op=AL.mult)
    expv = wpool.tile([P, KC, D], bf16, tag="expv")
    nc.scalar.activation(expv[:], varg[:], AF.Exp)
    vab = wpool.tile([P, KC, D], bf16, tag="vab")
    nc.gpsimd.tensor_tensor(out=vab[:], in0=expv[:], in1=abrow, op=AL.mult)
    nmul = wpool.tile([P, KC, D], bf16, tag="nmul")
    nc.gpsimd.tensor_tensor(out=nmul[:], in0=expn[:], in1=acolT, op=AL.mult)

    # ---- decays + affine scan over chunks ----
    decays = wpool.tile([D, 320], f32, tag="decays")
    nc.scalar.activation(decays[:, 0:160], bdtb_true.rearrange(
        "p m k -> p (m k)"), AF.Exp, scale=-1.0)
    nc.scalar.activation(decays[:, 160:320], bdtb_k0, AF.Exp, scale=-1.0)
    dw = wpool.tile([D, D, KC], f32, tag="dw")
    nc.vector.tensor_tensor(
        out=dw[:], in0=decays[:, 0:160].rearrange("p (m k) -> p m k", m=D),
        in1=wsq[:], op=AL.mult)
    # S_{k+1} = d0op_k * S_k + d_k*w_k  (d0op kills state at k=0 per m)
    Sout = wpool.tile([D, D * KC], bf16, tag="Sout")
    nc.vector.tensor_tensor_scan(
        Sout[:], decays[:, 160:320],
        dw[:].rearrange("p m k -> p (m k)"), initial=0.0,
        op0=AL.mult, op1=AL.add)
    Soutv = Sout[:].rearrange("p (m k) -> p m k", m=D)

    # ---- in-chunk inclusive prefix (PE) ----
    Pg = [pp.tile([P, 4, D, D], f32, tag=f"Pg{g}", name=f"Pg{g}")
          for g in range(NG)]
    for g in range(NG):
        nc.tensor.matmul(Pg[g][:], triu,
                         W[:, 4 * g:4 * (g + 1)].rearrange(
                             "p c r m -> p (c r m)"),
                         start=True, stop=True)

    # ---- gather inter-chunk state per event: Sg[i,m] = S_k[e_i, m] ----
    Sgall = pp.tile([P, KC, D], f32, tag="Sgall", name="Sgall")
    nc.vector.memset(Sgall[:, 0:1, :], 0.0)
    for k in range(1, KC):
        nc.tensor.matmul(Sgall[:, k, :], oht[:, k * P:(k + 1) * P],
                         Soutv[:, :, k - 1], start=True, stop=True)

    # ---- tail: per-group Act copy -> 2x product -> X-reduce over m ----
    PgSB = wpool.tile([P, KC, D, D], bf16, tag="PgSB")
    t1 = wpool.tile([P, KC, D, D], bf16, tag="t1")
    QS = wpool.tile([P, KC, 2, D], f32, tag="QS")
    negred = wpool.tile([P, KC], f32, tag="negred")
    negsub = wpool.tile([P, KC], f32, tag="negsub")
    for g in range(NG):
        gs = slice(4 * g, 4 * (g + 1))
        nc.scalar.copy(PgSB[:, gs], Pg[g][:])
        nc.vector.tensor_tensor(
            out=t1[:, gs], in0=PgSB[:, gs],
            in1=vab[:, gs].unsqueeze(2).broadcast_to([P, 4, D, D]),
            op=AL.mult)
        nc.vector.tensor_reduce(out=QS[:, gs, 0, :], in_=t1[:, gs],
                                axis=AX.X, op=AL.add)
        if g == 0:
            # S contribution + negative-part reduce in the g0/g1 bubble
            nc.vector.tensor_tensor(out=QS[:, :, 1, :], in0=vab[:],
                                    in1=Sgall[:], op=AL.mult)
        elif g == 1:
            nc.vector.tensor_reduce(out=negred[:], in_=nmul[:],
                                    axis=AX.X, op=AL.add)
            nc.vector.tensor_tensor(out=negsub[:], in0=negred[:],
                                    in1=asum_ev, op=AL.subtract)
    # mask r-lane by onehot_r, S-lane by ones, contract both at once
    t2 = wpool.tile([P, KC, 2, D], f32, tag="t2")
    nc.vector.tensor_tensor(out=t2[:], in0=QS[:], in1=ohone, op=AL.mult)
    lamr = wpool.tile([P, KC], f32, tag="lamr")
    nc.vector.tensor_reduce(
        out=lamr[:], in_=t2[:].rearrange("p c s m -> p c (s m)"),
        axis=AX.X, op=AL.add)

    lam = wpool.tile([P, KC], f32, tag="lam")
    nc.vector.tensor_tensor(out=lam[:], in0=lamr[:], in1=musub_ev,
                            op=AL.add)
    loglam = wpool.tile([P, KC], f32, tag="loglam")
    nc.scalar.activation(loglam[:], lam[:], AF.Ln)
    pev = wpool.tile([P, KC], f32, tag="pev")
    nc.vector.tensor_tensor(out=pev[:], in0=loglam[:], in1=negsub[:],
                            op=AL.add)
    nc.sync.dma_start(out=out_ap, in_=pev[:])


_CACHE = {}


def _build(Tval: float = 0.0):
    key = 0
    if key in _CACHE:
        return _CACHE[key]
    nc = bacc.Bacc("TRN2", target_bir_lowering=False, debug=False)
    ins = {}
    for name, (shape, dt) in INPUTS.items():
        ins[name] = nc.dram_tensor(name, list(shape), dt,
                                   kind="ExternalInput").ap()
    out_ap = nc.dram_tensor("out", [P, KC], f32, kind="ExternalOutput").ap()
    with tile.TileContext(nc) as tc:
        with ExitStack() as ctx:
            _body(ctx, tc, ins, out_ap)
    nc.compile()
    _CACHE[key] = (nc, ins, out_ap)
    return _CACHE[key]


def make_in_maps(time_points, event_types, mu_raw, log_alpha, log_beta, T):
    Tval = float(np.asarray(T))
    tp = np.asarray(time_points, dtype=np.float32)          # [B, N]
    et = np.asarray(event_types).astype(np.int64)           # [B, N]

    # O(D^2) parameter transforms in float64 -> float32
    mu = np.log1p(np.exp(np.float64(mu_raw))).astype(np.float32)
    al = np.log1p(np.exp(np.float64(log_alpha))).astype(np.float32)
    be = np.log1p(np.exp(np.float64(log_beta))).astype(np.float32)
    ab = (al * be).astype(np.float32)
    musub = mu - np.diag(ab)                                # [D]
    asum = al.sum(axis=0)                                   # [D]
    beT = np.ascontiguousarray(be.T)
    alT = np.ascontiguousarray(al.T)

    triu = np.triu(np.ones((P, P), dtype=np.float32))

    in_maps = []
    for b in range(B):
        e = et[b]                                           # [N]
        t = tp[b]
        ts = t[::P]                                         # [KC]
        dtb = np.zeros(KC, dtype=np.float32)
        dtb[:-1] = ts[1:] - ts[:-1]

        # [p, c] views (event j = c*128 + p)
        t2 = t.reshape(KC, P).T                             # [P, KC]
        e2 = e.reshape(KC, P).T                             # [P, KC]

        hot_f32 = np.zeros((P, 192), dtype=np.float32)
        hot_f32[:, 0:16] = t2 - ts[None, :]                 # trel
        hot_f32[:, 16:32] = t2 - np.float32(Tval)           # tau2
        hot_f32[:, 32:192] = beT[e2].reshape(P, KC * D)     # bcol

        oh = (e2[:, :, None] == np.arange(D)[None, None, :])
        hot_bf = np.zeros((P, 288), dtype=ml_dtypes.bfloat16)
        hot_bf[:, 0:160] = oh.reshape(P, KC * D)
        hot_bf[:, 160:288] = triu

        rest_f32 = np.zeros((P, 208), dtype=np.float32)
        rest_f32[:, 0:160] = (-be)[e2].reshape(P, KC * D)   # brow_neg
        rest_f32[:, 160:176] = musub[e2]
        rest_f32[:, 176:192] = asum[e2]

        rest_bf = np.zeros((P, 640), dtype=ml_dtypes.bfloat16)
        rest_bf[:, 0:160] = alT[e2].reshape(P, KC * D)      # acolT
        rest_bf[:, 160:320] = ab[e2].reshape(P, KC * D)     # abrow
        ohone = np.zeros((P, KC, 2, D), dtype=np.float32)
        ohone[:, :, 0, :] = oh
        ohone[:, :, 1, :] = 1.0
        rest_bf[:, 320:640] = ohone.reshape(P, 320)

        oht = np.zeros((D, N + 320), dtype=ml_dtypes.bfloat16)
        oht[:, 0:N] = (e[None, :] == np.arange(D)[:, None])
        bdtb = be[:, :, None] * dtb[None, None, :]          # [D, D, KC]
        oht[:, N:N + 160] = bdtb.reshape(D, D * KC)
        bk0 = bdtb.copy()
        bk0[:, :, 0] = 40.0                                 # exp(-40) ~ 0
        oht[:, N + 160:N + 320] = bk0.reshape(D, D * KC)

        in_maps.append({"hot_f32": hot_f32, "hot_bf": hot_bf,
                        "rest_f32": rest_f32, "rest_bf": rest_bf,
                        "oht": oht})
    negconst = np.float32(-Tval * mu.astype(np.float64).sum())
    return in_maps, Tval, negconst


def kernel(time_points, event_types, mu_raw, log_alpha, log_beta, T):
    in_maps, Tval, negconst = make_in_maps(
        time_points, event_types, mu_raw, log_alpha, log_beta, T)
    nc, _, _ = _build(Tval)
    res = run_bass_kernel_spmd(nc, in_maps, list(range(B))).results
    out = np.array([res[b]["out"].sum() + negconst for b in range(B)],
                   dtype=np.float32)
    return out


# revision 8
# speedup vs baseline: 1.0719x; 1.0719x over previous
"""Trainium2 Bass kernel for the exp-kernel multivariate Hawkes process
log-likelihood (B=8, N=2048, D=10).

Strategy (v6)
-------------
Data-parallel over batch: core b computes batch row b fully on-chip and
returns pev[128,16] per-event partials; the host reduces them and adds
the -T*sum(mu) constant (unshard step).

Host ships per-event GATHERED tables (pure index lookups, O(N*D)):
  bcol[j,r]  = b[r, e_j]        brow_neg[j,m] = -b[e_j, m]
  acolT[j,d] = a[d, e_j]        abrow[j,m]    = (a*b)[e_j, m]
so the device only exponentiates [N,10] grids:
  u[j,r]    = exp(bcol * trel_j)       expn = exp(bcol * tau2)
  vab[i,m]  = exp(brow_neg * trel_i) * abrow
The u exp and the [128, KC*100] pair grid W = u (x) onehot_m are built
in halves so the prefix matmuls start as early as possible.  The
secondary products (varg, vab, nmul) run on the otherwise-idle GPSIMD
engine so they can never head-of-line-block the DVE critical chain.

Tail (factored, no second pair grid):
  lam_core[i] = sum_r onehot_r[i,r] * (sum_m PgSB[i,(r,m)] * vab[i,m])
via per-group 2x products against Act-engine bf16 copies of the PSUM
prefix + X-reduces, pipelined group-by-group against the copies.  The
inter-chunk S contribution and the r-mask share one concatenated
[P,KC,2,D] mask pass.

Inter-chunk state S lives in [10_r, (10_m, 16_k)] layout end-to-end:
chunk sums from 16 tiny u^T@onehot matmuls (strided PSUM writes), the
affine recurrence S_{k+1}=d_k(S_k+w_k) is ONE tensor_tensor_scan with
a k=0 decay-reset column, and S is gathered per event with 15 onehotT
matmuls.  A manually emitted InstLoadActFuncSet(id=6) loads the
combined exp+ln table once.

Host-side work is limited to O(D^2) parameter softplus and O(N*D)
gathers/reshapes/sums of index tensors (no transcendental math on N).
"""
import numpy as np
from contextlib import ExitStack

import ml_dtypes
import concourse.bass as bass
import concourse.mybir as mybir
import concourse.tile as tile
from concourse import bacc
from concourse.bass_utils import run_bass_kernel_spmd

f32 = mybir.dt.float32
bf16 = mybir.dt.bfloat16
AL = mybir.AluOpType
AF = mybir.ActivationFunctionType
AX = mybir.AxisListType

P = 128          # partitions == chunk size
KC = 16          # number of chunks
D = 10           # event types
RM = D * D       # (receiver, trigger) pairs
N = P * KC       # 2048 events per batch row
B = 8            # batch == cores
NG = 4           # chunk groups (4 chunks per PSUM bank)

# packed DRAM inputs: name -> (shape, dtype)
INPUTS = {
    "hot_f32": ((P, 192), f32),    # trel(16) tau2(16) bcol(160)
    "hot_bf": ((P, 288), bf16),    # onehot(160) triu(128)
    "rest_f32": ((P, 208), f32),   # brow_neg(160) musub_ev(16) asum_ev(16)
                                   # pad(16)
    "rest_bf": ((P, 640), bf16),   # acolT(160) abrow(160) ohone(320)
    "oht": ((D, N + 320), bf16),   # onehotT [D,N] | decay args true |
                                   # decay args k0-killed
}


def _body(ctx: ExitStack, tc, ins, out_ap):
    nc = tc.nc
    cpool = ctx.enter_context(tc.tile_pool(name="cpool", bufs=1))
    wpool = ctx.enter_context(tc.tile_pool(name="wpool", bufs=1))
    pp = ctx.enter_context(tc.tile_pool(name="pp", bufs=1, space="PSUM"))

    # one combined exp+ln activation table load (id 6 =
    # natural_log_exp_and_others) emitted first on the Act queue
    nc.scalar.add_instruction(mybir.InstLoadActFuncSet(
        name=nc.get_next_instruction_name(), act_func_set_id=6,
        ins=[], outs=[]))

    # ---- input DMAs: hot tiles first on the sync/scalar queues ----
    hot_f32 = cpool.tile([P, 192], f32, tag="hot_f32")
    nc.sync.dma_start(out=hot_f32[:], in_=ins["hot_f32"])
    hot_bf = cpool.tile([P, 288], bf16, tag="hot_bf")
    nc.scalar.dma_start(out=hot_bf[:], in_=ins["hot_bf"])
    rest_f32 = cpool.tile([P, 208], f32, tag="rest_f32")
    nc.sync.dma_start(out=rest_f32[:], in_=ins["rest_f32"])
    rest_bf = cpool.tile([P, 640], bf16, tag="rest_bf")
    nc.scalar.dma_start(out=rest_bf[:], in_=ins["rest_bf"])
    oht = cpool.tile([D, N + 320], bf16, tag="oht")
    nc.sync.dma_start(out=oht[:], in_=ins["oht"])

    trel = hot_f32[:, 0:16]
    tau2 = hot_f32[:, 16:32]
    bcol = hot_f32[:, 32:192].rearrange("p (c r) -> p c r", c=KC)
    onehot = hot_bf[:, 0:160].rearrange("p (c m) -> p c m", c=KC)
    triu = hot_bf[:, 160:288]
    brow_neg = rest_f32[:, 0:160].rearrange("p (c m) -> p c m", c=KC)
    musub_ev = rest_f32[:, 160:176]
    asum_ev = rest_f32[:, 176:192]
    acolT = rest_bf[:, 0:160].rearrange("p (c d) -> p c d", c=KC)
    abrow = rest_bf[:, 160:320].rearrange("p (c m) -> p c m", c=KC)
    ohone = rest_bf[:, 320:640].rearrange("p (c s m) -> p c s m", c=KC, s=2)

    # ---- critical chain: au -> eu (halves) -> W (halves) -> prefix ----
    au = wpool.tile([P, KC, D], f32, tag="au")
    nc.vector.tensor_tensor(
        out=au[:], in0=bcol,
        in1=trel.unsqueeze(2).broadcast_to([P, KC, D]), op=AL.mult)
    u = wpool.tile([P, KC, D], bf16, tag="u")
    nc.scalar.activation(u[:, 0:8], au[:, 0:8], AF.Exp)
    nc.scalar.activation(u[:, 8:16], au[:, 8:16], AF.Exp)
    W = wpool.tile([P, KC, D, D], bf16, tag="W")
    for h in range(2):
        hs = slice(8 * h, 8 * (h + 1))
        nc.vector.tensor_tensor(
            out=W[:, hs],
            in0=u[:, hs].unsqueeze(3).broadcast_to([P, 8, D, D]),
            in1=onehot[:, hs].unsqueeze(2).broadcast_to([P, 8, D, D]),
            op=AL.mult)

    # negative-part exp arg (DVE bubble while Act runs eu halves)
    an = wpool.tile([P, KC, D], f32, tag="an")
    nc.vector.tensor_tensor(
        out=an[:], in0=bcol,
        in1=tau2.unsqueeze(2).broadcast_to([P, KC, D]), op=AL.mult)
    expn = wpool.tile([P, KC, D], bf16, tag="expn")
    nc.scalar.activation(expn[:], an[:], AF.Exp)

    # chunk sums straight into scan layout: wsq[r, m, k]
    wsq = pp.tile([D, D, KC], f32, tag="wsq", name="wsq")
    for k in range(KC):
        nc.tensor.matmul(wsq[:, :, k], u[:, k, :], onehot[:, k, :],
                         start=True, stop=True)

    # ---- secondary exp pipeline: products on GPSIMD ----
    varg = wpool.tile([P, KC, D], f32, tag="varg")
    nc.gpsimd.tensor_tensor(
        out=varg[:], in0=brow_neg,
        in1=trel.unsqueeze(2).broadcast_to([P, KC, D]), op=AL.mult)
    expv = wpool.tile([P, KC, D], bf16, tag="expv")
    nc.scalar.activation(expv[:], varg[:], AF.Exp)
    vab = wpool.tile([P, KC, D], bf16, tag="vab")
    nc.gpsimd.tensor_tensor(out=vab[:], in0=expv[:], in1=abrow, op=AL.mult)
    nmul = wpool.tile([P, KC, D], bf16, tag="nmul")
    nc.gpsimd.tensor_tensor(out=nmul[:], in0=expn[:], in1=acolT, op=AL.mult)

    # ---- decays + affine scan over chunks ----
    decays = wpool.tile([D, 320], f32, tag="decays")
    nc.scalar.activation(decays[:], oht[:, N:N + 320], AF.Exp, scale=-1.0)
    dw = wpool.tile([D, D, KC], f32, tag="dw")
    nc.vector.tensor_tensor(
        out=dw[:], in0=decays[:, 0:160].rearrange("p (m k) -> p m k", m=D),
        in1=wsq[:], op=AL.mult)
    # S_{k+1} = d0op_k * S_k + d_k*w_k  (d0op kills state at k=0 per m)
    Sout = wpool.tile([D, D * KC], bf16, tag="Sout")
    nc.vector.tensor_tensor_scan(
        Sout[:], decays[:, 160:320],
        dw[:].rearrange("p m k -> p (m k)"), initial=0.0,
        op0=AL.mult, op1=AL.add)
    Soutv = Sout[:].rearrange("p (m k) -> p m k", m=D)

    # ---- in-chunk inclusive prefix (PE) ----
    Pg = [pp.tile([P, 4, D, D], f32, tag=f"Pg{g}", name=f"Pg{g}")
          for g in range(NG)]
    for g in range(NG):
        nc.tensor.matmul(Pg[g][:], triu,
                         W[:, 4 * g:4 * (g + 1)].rearrange(
                             "p c r m -> p (c r m)"),
                         start=True, stop=True)

    # ---- gather inter-chunk state per event: Sg[i,m] = S_k[e_i, m] ----
    Sgall = pp.tile([P, KC, D], f32, tag="Sgall", name="Sgall")
    nc.vector.memset(Sgall[:, 0:1, :], 0.0)
    for k in range(1, KC):
        nc.tensor.matmul(Sgall[:, k, :], oht[:, k * P:(k + 1) * P],
                         Soutv[:, :, k - 1], start=True, stop=True)

    # ---- tail: per-group Act copy -> 2x product -> X-reduce over m ----
    PgSB = wpool.tile([P, KC, D, D], bf16, tag="PgSB")
    t1 = wpool.tile([P, KC, D, D], bf16, tag="t1")
    QS = wpool.tile([P, KC, 2, D], bf16, tag="QS")
    negred = wpool.tile([P, KC], f32, tag="negred")
    lamns = wpool.tile([P, 32], f32, tag="lamns")
    for g in range(NG):
        gs = slice(4 * g, 4 * (g + 1))
        nc.scalar.copy(PgSB[:, gs], Pg[g][:])
        nc.vector.tensor_tensor(
            out=t1[:, gs], in0=PgSB[:, gs],
            in1=vab[:, gs].unsqueeze(2).broadcast_to([P, 4, D, D]),
            op=AL.mult)
        with nc.allow_low_precision("Q partials; masked+summed, 2e-2 budget"):
            nc.vector.tensor_reduce(out=QS[:, gs, 0, :], in_=t1[:, gs],
                                    axis=AX.X, op=AL.add)
        if g == 0:
            # S contribution + negative-part reduce in the g0/g1 bubble
            nc.vector.tensor_tensor(out=QS[:, :, 1, :], in0=vab[:],
                                    in1=Sgall[:], op=AL.mult)
        elif g == 1:
            nc.vector.tensor_reduce(out=negred[:], in_=nmul[:],
                                    axis=AX.X, op=AL.add)
            nc.vector.tensor_tensor(out=lamns[:, 16:32],
                                    in0=negred[:], in1=asum_ev,
                                    op=AL.subtract)
    # mask r-lane by onehot_r, S-lane by ones, contract both at once
    t2 = wpool.tile([P, KC, 2, D], bf16, tag="t2")
    nc.vector.tensor_tensor(out=t2[:], in0=QS[:], in1=ohone, op=AL.mult)
    lamr = wpool.tile([P, KC], f32, tag="lamr")
    nc.vector.tensor_reduce(
        out=lamr[:], in_=t2[:].rearrange("p c s m -> p c (s m)"),
        axis=AX.X, op=AL.add)

    lam = wpool.tile([P, KC], f32, tag="lam")
    nc.vector.tensor_tensor(out=lam[:], in0=lamr[:], in1=musub_ev,
                            op=AL.add)
    nc.scalar.activation(lamns[:, 0:16], lam[:], AF.Ln)
    nc.sync.dma_start(out=out_ap, in_=lamns[:])


_CACHE = {}


def _build(Tval: float = 0.0):
    key = 0
    if key in _CACHE:
        return _CACHE[key]
    nc = bacc.Bacc("TRN2", target_bir_lowering=False, debug=False)
    ins = {}
    for name, (shape, dt) in INPUTS.items():
        ins[name] = nc.dram_tensor(name, list(shape), dt,
                                   kind="ExternalInput").ap()
    out_ap = nc.dram_tensor("out", [P, 32], f32,
                            kind="ExternalOutput").ap()
    with tile.TileContext(nc) as tc:
        with ExitStack() as ctx:
            _body(ctx, tc, ins, out_ap)
    nc.compile()
    _CACHE[key] = (nc, ins, out_ap)
    return _CACHE[key]


def make_in_maps(time_points, event_types, mu_raw, log_alpha, log_beta, T):
    Tval = float(np.asarray(T))
    tp = np.asarray(time_points, dtype=np.float32)          # [B, N]
    et = np.asarray(event_types).astype(np.int64)           # [B, N]

    # O(D^2) parameter transforms in float64 -> float32
    mu = np.log1p(np.exp(np.float64(mu_raw))).astype(np.float32)
    al = np.log1p(np.exp(np.float64(log_alpha))).astype(np.float32)
    be = np.log1p(np.exp(np.float64(log_beta))).astype(np.float32)
    ab = (al * be).astype(np.float32)
    musub = mu - np.diag(ab)                                # [D]
    asum = al.sum(axis=0)                                   # [D]
    beT = np.ascontiguousarray(be.T)
    alT = np.ascontiguousarray(al.T)

    triu = np.triu(np.ones((P, P), dtype=np.float32))

    in_maps = []
    for b in range(B):
        e = et[b]                                           # [N]
        t = tp[b]
        ts = t[::P]                                         # [KC]
        dtb = np.zeros(KC, dtype=np.float32)
        dtb[:-1] = ts[1:] - ts[:-1]

        # [p, c] views (event j = c*128 + p)
        t2 = t.reshape(KC, P).T                             # [P, KC]
        e2 = e.reshape(KC, P).T                             # [P, KC]

        hot_f32 = np.zeros((P, 192), dtype=np.float32)
        hot_f32[:, 0:16] = t2 - ts[None, :]                 # trel
        hot_f32[:, 16:32] = t2 - np.float32(Tval)           # tau2
        hot_f32[:, 32:192] = beT[e2].reshape(P, KC * D)     # bcol

        oh = (e2[:, :, None] == np.arange(D)[None, None, :])
        hot_bf = np.zeros((P, 288), dtype=ml_dtypes.bfloat16)
        hot_bf[:, 0:160] = oh.reshape(P, KC * D)
        hot_bf[:, 160:288] = triu

        rest_f32 = np.zeros((P, 208), dtype=np.float32)
        rest_f32[:, 0:160] = (-be)[e2].reshape(P, KC * D)   # brow_neg
        rest_f32[:, 160:176] = musub[e2]
        rest_f32[:, 176:192] = asum[e2]

        rest_bf = np.zeros((P, 640), dtype=ml_dtypes.bfloat16)
        rest_bf[:, 0:160] = alT[e2].reshape(P, KC * D)      # acolT
        rest_bf[:, 160:320] = ab[e2].reshape(P, KC * D)     # abrow
        ohone = np.zeros((P, KC, 2, D), dtype=np.float32)
        ohone[:, :, 0, :] = oh
        ohone[:, :, 1, :] = 1.0
        rest_bf[:, 320:640] = ohone.reshape(P, 320)

        oht = np.zeros((D, N + 320), dtype=ml_dtypes.bfloat16)
        oht[:, 0:N] = (e[None, :] == np.arange(D)[:, None])
        bdtb = be[:, :, None] * dtb[None, None, :]          # [D, D, KC]
        oht[:, N:N + 160] = bdtb.reshape(D, D * KC)
        bk0 = bdtb.copy()
        bk0[:, :, 0] = 40.0                                 # exp(-40) ~ 0
        oht[:, N + 160:N + 320] = bk0.reshape(D, D * KC)

        in_maps.append({"hot_f32": hot_f32, "hot_bf": hot_bf,
                        "rest_f32": rest_f32, "rest_bf": rest_bf,
                        "oht": oht})
    negconst = np.float32(-Tval * mu.astype(np.float64).sum())
    return in_maps, Tval, negconst


def kernel(time_points, event_types, mu_raw, log_alpha, log_beta, T):
    in_maps, Tval, negconst = make_in_maps(
        time_points, event_types, mu_raw, log_alpha, log_beta, T)
    nc, _, _ = _build(Tval)
    res = run_bass_kernel_spmd(nc, in_maps, list(range(B))).results
    out = np.array([res[b]["out"].sum() + negconst for b in range(B)],
                   dtype=np.float32)  # loglam + negsub halves both summed
    return out
